# revision 4
# baseline (speedup 1.0000x reference)
"""Trainium2 Bass kernel for nn_CategoricalRegressionLoss (C51 categorical
projection cross-entropy loss).

Math (per row b, 51 atoms):
    p      = softmax(logits_tp1)                       # [51]
    tz     = clip(atoms_target_t, -10, 10)             # [51]
    y      = (tz + 10) / 0.4            in [0, 50]     # atom-grid coords
    G_b(y) = sum_i x[b,i] * relu(1 - |y - i|)          # PWL interp of x = logits_t
    ce[b]  = logsumexp(x[b,:]) - sum_j p[b,j] * G_b(y[b,j])
    out    = mean_b ce[b]

The projection/cross-entropy contraction collapses into a single bilinear
form over the expanded (j, i) grid per row:
    sum_j p_j G_b(y_j) = sum_{j,i} relu(1 - |y_j - i|) * (p_j * x_i)
which we evaluate densely on-chip:
    d   = y_j - i                  (TT subtract, broadcast APs)
    e   = min(|d|, 1)              (tensor_scalar abs_max/min, 2x mode)
    Q   = p_j * x_i                (TT outer product)
    acc = sum (e - 1) * Q          (scalar_tensor_tensor with accum_out)
    => sum_j p_j G_b = -acc

Sharding: pure data parallel, batch 65536 -> 8 cores x 8192 rows.
Each core computes a partial sum of ce over its rows; host sums and
divides by the batch size.
"""

import sys

sys.path.insert(0, "/opt/trn_rl_repo")

import numpy as np

import concourse.bacc as bacc
import concourse.tile as tile
import concourse.mybir as mybir
from concourse.bass_utils import run_bass_kernel_spmd

N_CORES = 8
BS = 65536
NA = 51  # num atoms
NI = 52  # padded atom axis (extra zero atom keeps inner dim even)
R = BS // N_CORES  # rows per core = 8192
P = 128  # partitions
G = R // P  # row-groups per core = 64

F32 = mybir.dt.float32
ALU = mybir.AluOpType
ACT = mybir.ActivationFunctionType

_CACHE = {}


def _build():
    nc = bacc.Bacc("TRN2", target_bir_lowering=False)

    lt = nc.dram_tensor("logits_t", (R, NA), F32, kind="ExternalInput")
    lp = nc.dram_tensor("logits_tp1", (R, NA), F32, kind="ExternalInput")
    at = nc.dram_tensor("atoms_target_t", (R, NA), F32, kind="ExternalInput")
    out = nc.dram_tensor("out", (1, 1), F32, kind="ExternalOutput")

    # row r = p*G + g  ->  partition p, group g (per-partition contiguous DMA)
    lt_r = lt.rearrange("(p g) a -> p g a", p=P)
    lp_r = lp.rearrange("(p g) a -> p g a", p=P)
    at_r = at.rearrange("(p g) a -> p g a", p=P)

    with tile.TileContext(nc) as tc:
        with (
            tc.tile_pool(name="mega", bufs=1) as mega,
            tc.tile_pool(name="small", bufs=1) as small,
            tc.tile_pool(name="exp", bufs=3) as expp,
            tc.tile_pool(name="expq", bufs=3) as expq,
            tc.tile_pool(name="psum", bufs=1, space="PSUM") as psum,
        ):
            # ---- constants ----
            ipat_i = small.tile([P, NI], mybir.dt.int32)
            nc.gpsimd.iota(ipat_i, pattern=[[1, NI]], base=0, channel_multiplier=0)
            ipat = small.tile([P, NI], F32)
            nc.vector.tensor_copy(ipat, ipat_i)
            ones_col = small.tile([P, 1], F32)
            nc.vector.memset(ones_col, 1.0)

            # ---- load inputs ----
            xe = mega.tile([P, G, NI], F32)  # logits_t padded with zero col
            nc.vector.memset(xe, 0.0)
            nc.sync.dma_start(out=xe[:, :, 0:NA], in_=lt_r)
            tlp = mega.tile([P, G, NA], F32)
            nc.sync.dma_start(out=tlp, in_=lp_r)
            tat = mega.tile([P, G, NA], F32)
            nc.sync.dma_start(out=tat, in_=at_r)

            x = xe[:, :, 0:NA]

            # ---- phase 1: softmax(p), logsumexp, y ----
            eT = mega.tile([P, G, NA], F32)
            nc.scalar.activation(eT, x, ACT.Exp)
            sT = small.tile([P, G], F32)
            nc.vector.tensor_reduce(sT, eT, axis=mybir.AxisListType.X, op=ALU.add)
            lse = small.tile([P, G], F32)
            nc.scalar.activation(lse, sT, ACT.Ln)

            eP = mega.tile([P, G, NA], F32)
            nc.scalar.activation(eP, tlp, ACT.Exp)
            sP = small.tile([P, G], F32)
            nc.vector.tensor_reduce(sP, eP, axis=mybir.AxisListType.X, op=ALU.add)
            rP = small.tile([P, G], F32)
            nc.vector.reciprocal(rP, sP)
            # p = eP * rP (broadcast over atoms), in place
            nc.vector.tensor_tensor(
                eP, eP, rP.unsqueeze(2).broadcast_to((P, G, NA)), ALU.mult
            )

            # y = (clip(at, -10, 10) + 10) / 0.4 = clip*2.5 + 25, in place
            nc.vector.tensor_scalar(
                out=tat, in0=tat, scalar1=10.0, scalar2=-10.0, op0=ALU.min, op1=ALU.max
            )
            nc.vector.tensor_scalar(
                out=tat, in0=tat, scalar1=2.5, scalar2=25.0, op0=ALU.mult, op1=ALU.add
            )

            accAll = small.tile([P, G], F32)
            sqAll = small.tile([P, G], F32)

            # ---- phase 2: expanded (j, i) contraction per row-group ----
            # sum_j p_j G_b(y_j) = sum_{j,i} relu(1-|y_j-i|) p_j x_i
            #                    = sum Q - sum min(|d|,1)*Q,   Q = p_j x_i
            for g in range(G):
                yB = tat[:, g, :].unsqueeze(2).broadcast_to((P, NA, NI))
                iB = ipat.unsqueeze(1).broadcast_to((P, NA, NI))
                d = expp.tile([P, NA, NI], F32)
                nc.vector.tensor_tensor(d, yB, iB, ALU.subtract)
                nc.scalar.activation(d, d, ACT.Abs)  # |d| (ACT, hidden)
                # Q = p_j * x_i outer product, with accum sQ = sum Q
                pB = eP[:, g, :].unsqueeze(2).broadcast_to((P, NA, NI))
                xB = xe[:, g, :].unsqueeze(1).broadcast_to((P, NA, NI))
                q = expq.tile([P, NA, NI], F32)
                nc.vector.scalar_tensor_tensor(
                    out=q,
                    in0=pB,
                    scalar=1.0,
                    in1=xB,
                    op0=ALU.mult,
                    op1=ALU.mult,
                    accum_out=sqAll[:, g : g + 1],
                )
                # acc = sum min(|d|,1) * Q
                nc.vector.scalar_tensor_tensor(
                    out=q,
                    in0=d,
                    scalar=1.0,
                    in1=q,
                    op0=ALU.min,
                    op1=ALU.mult,
                    accum_out=accAll[:, g : g + 1],
                )

            # ---- tail: ce = lse - sQ + acc ; partial = sum(ce) ----
            ce = small.tile([P, G], F32)
            nc.vector.tensor_tensor(ce, lse, sqAll, ALU.subtract)
            nc.vector.tensor_tensor(ce, ce, accAll, ALU.add)
            ctot = small.tile([P, 1], F32)
            nc.vector.tensor_reduce(ctot, ce, axis=mybir.AxisListType.X, op=ALU.add)

            ps = psum.tile([1, 1], F32)
            nc.tensor.matmul(ps, lhsT=ctot, rhs=ones_col, start=True, stop=True)
            res = small.tile([1, 1], F32)
            nc.scalar.copy(res, ps)
            nc.sync.dma_start(out=out[:, :], in_=res)

    nc.compile()
    return nc


def kernel(logits_t, logits_tp1, atoms_target_t):
    if "nc" not in _CACHE:
        _CACHE["nc"] = _build()
    nc = _CACHE["nc"]

    logits_t = np.ascontiguousarray(logits_t, dtype=np.float32)
    logits_tp1 = np.ascontiguousarray(logits_tp1, dtype=np.float32)
    atoms_target_t = np.ascontiguousarray(atoms_target_t, dtype=np.float32)

    in_maps = []
    for k in range(N_CORES):
        sl = slice(k * R, (k + 1) * R)
        in_maps.append(
            {
                "logits_t": logits_t[sl],
                "logits_tp1": logits_tp1[sl],
                "atoms_target_t": atoms_target_t[sl],
            }
        )

    res = run_bass_kernel_spmd(nc, in_maps, core_ids=list(range(N_CORES)))
    total = sum(float(res.results[k]["out"][0, 0]) for k in range(N_CORES))
    return np.float32(total / BS)


# revision 12
# speedup vs baseline: 197.3336x; 197.3336x over previous
"""Trainium2 Bass kernel for nn_CategoricalRegressionLoss (C51 categorical
projection cross-entropy loss).

Math (per row b, 51 atoms, x = logits_t):
    p      = softmax(logits_tp1)
    y      = (clip(atoms_target_t, -10, 10) + 10) / 0.4     in [0, 50]
    G_b(y) = sum_i x[b,i] * relu(1 - |y - i|)     (PWL interp of x at y)
    ce[b]  = logsumexp(x[b,:]) - sum_j p[b,j] * G_b(y[b,j])
    out    = mean_b ce[b]

Dense evaluation over the expanded (j, i) grid:
    sum_j p_j G_b(y_j) = sum Q - sum min(|d|,1)*Q
    d = y_j - i,  Q = p_j * x_i,  sum Q = rowsum(x) * sum(p)

Engine split per 128-row group g:
    PE     d = y_j - i: transpose [y_hi|y_lo|1] (exact bf16 split of y), then
           one bf16 matmul per PSUM chunk against a selection matrix
           (rows j' place y at (j=j', i) blocks; last row adds -i).
    ACT    |d| PSUM->SBUF(bf16), exp/ln in phase 1
    DVE    fused min/mul/accumulate pass (bf16 2x) + 1/4 of Q builds
    GPSIMD Q = p_j * x_i outer products (3/4 of groups)

Sharding: pure data parallel, batch 65536 -> 8 cores x 8192 rows. Each core
emits a partial ce sum; host sums / batch size.
"""

import sys

sys.path.insert(0, "/opt/trn_rl_repo")

import numpy as np

import concourse.bacc as bacc
import concourse.tile as tile
import concourse.mybir as mybir
from concourse.bass_utils import run_bass_kernel_spmd
from concourse.masks import make_identity

N_CORES = 8
BS = 65536
NA = 51  # num atoms
NI = 52  # padded atom axis (even inner dim; extra atom has zero weight)
NK = 103  # contraction: 51 y_hi + 51 y_lo + ones row
R = BS // N_CORES  # rows per core
P = 128
G = R // P  # row-groups per core = 64

# PSUM d-chunks: 51 j-groups of 52 cols, ping-ponged over two 3-bank pools
CH_A = [(0, 9), (9, 9), (18, 9)]  # j 0..26
CH_B = [(27, 9), (36, 9), (45, 9)]  # j 27..53 (j 51..53 are zero pad)
NJ = 54  # padded j axis

F32 = mybir.dt.float32
BF16 = mybir.dt.bfloat16
I32 = mybir.dt.int32
ALU = mybir.AluOpType
ACT = mybir.ActivationFunctionType
AX = mybir.AxisListType

QDVE_EVERY = 4  # every 4th group's Q built on DVE, rest on GPSIMD

_CACHE = {}


def _build():
    nc = bacc.Bacc("TRN2", target_bir_lowering=False)

    lt = nc.dram_tensor("logits_t", (R, NA), F32, kind="ExternalInput")
    lp = nc.dram_tensor("logits_tp1", (R, NA), F32, kind="ExternalInput")
    at = nc.dram_tensor("atoms_target_t", (R, NA), F32, kind="ExternalInput")
    out = nc.dram_tensor("out", (1, 1), F32, kind="ExternalOutput")

    lt_r = lt.rearrange("(p g) a -> p g a", p=P)
    lp_r = lp.rearrange("(p g) a -> p g a", p=P)
    at_r = at.rearrange("(p g) a -> p g a", p=P)

    with tile.TileContext(nc) as tc:
        with (
            tc.tile_pool(name="mega", bufs=1) as mega,
            tc.tile_pool(name="small", bufs=1) as small,
            tc.tile_pool(name="lhp", bufs=3) as lhp,
            tc.tile_pool(name="expp", bufs=3) as expp,
            tc.tile_pool(name="expq", bufs=3) as expq,
            tc.tile_pool(name="psT", bufs=1, space="PSUM") as psT,
            tc.tile_pool(name="psDA", bufs=1, space="PSUM") as psDA,
            tc.tile_pool(name="psDB", bufs=1, space="PSUM") as psDB,
        ):
            # ---- constants ----
            identb = small.tile([P, P], BF16)
            make_identity(nc, identb)

            # selb[k, c, col]: for chunk c covering j'=9c..9c+8,
            # row j' (y_hi) and row 51+j' (y_lo) have ones on the 52-col block
            # of j'; row 102 has the -i pattern everywhere. Built with
            # full-tile iota + compares (partition-base-0 accesses only).
            NC6 = 6 * 468
            selb = small.tile([NK, 6, 512], BF16)
            nc.vector.memset(selb, 0.0)
            itA = small.tile([NK, 6, 468], I32)
            vA = itA.rearrange("p c (j i) -> p c j i", i=NI)
            nc.gpsimd.iota(
                vA, pattern=[[-9, 6], [-1, 9], [0, NI]], base=0, channel_multiplier=1
            )  # value = k - 9c - jl
            fA = small.tile([NK, 6, 468], F32)
            nc.vector.tensor_copy(fA, itA)
            itI = small.tile([NK, 6, 468], I32)
            vI = itI.rearrange("p c (j i) -> p c j i", i=NI)
            nc.gpsimd.iota(
                vI, pattern=[[0, 6], [0, 9], [-1, NI]], base=0, channel_multiplier=0
            )  # value = -i
            fI = small.tile([NK, 6, 468], F32)
            nc.vector.tensor_copy(fI, itI)
            itK = small.tile([NK, 6, 468], I32)
            nc.gpsimd.iota(
                itK[:, :, :], pattern=[[0, 6], [0, 468]], base=-102,
                channel_multiplier=1,
            )  # value = k - 102
            fK = small.tile([NK, 6, 468], F32)
            nc.vector.tensor_copy(fK, itK)
            # mh + ml: hi block (fA==0) + lo block (fA==51)
            mh = small.tile([NK, 6, 468], F32)
            nc.vector.tensor_scalar(
                out=mh, in0=fA, scalar1=0.0, scalar2=None, op0=ALU.is_equal
            )
            ml = small.tile([NK, 6, 468], F32)
            nc.vector.tensor_scalar(
                out=ml, in0=fA, scalar1=51.0, scalar2=None, op0=ALU.is_equal
            )
            nc.vector.tensor_tensor(mh, mh, ml, ALU.add)
            # row 102: (fK==0) * (-i)
            nc.vector.tensor_scalar(
                out=fK, in0=fK, scalar1=0.0, scalar2=None, op0=ALU.is_equal
            )
            nc.vector.tensor_tensor(fK, fK, fI, ALU.mult)
            nc.vector.tensor_tensor(
                selb[:, :, 0:468], mh, fK, ALU.add
            )
            ones_col = small.tile([P, 1], F32)
            nc.vector.memset(ones_col, 1.0)

            # ---- load inputs ----
            xe = mega.tile([P, G, NI], F32)  # logits_t, col 51 zero
            nc.vector.memset(xe[:, :, NA:NI], 0.0)
            nc.sync.dma_start(out=xe[:, :, 0:NA], in_=lt_r)
            tlp = mega.tile([P, G, NA], F32)
            nc.sync.dma_start(out=tlp, in_=lp_r)
            tat = mega.tile([P, G, NA], F32)
            nc.sync.dma_start(out=tat, in_=at_r)

            x = xe[:, :, 0:NA]

            # ---- phase 1 ----
            eT = mega.tile([P, G, NA], F32)
            nc.scalar.activation(eT, x, ACT.Exp)
            sT = small.tile([P, G], F32)
            nc.vector.tensor_reduce(sT, eT, axis=AX.X, op=ALU.add)
            lse = small.tile([P, G], F32)
            nc.scalar.activation(lse, sT, ACT.Ln)

            eP = mega.tile([P, G, NA], F32)
            nc.scalar.activation(eP, tlp, ACT.Exp)
            sP = small.tile([P, G], F32)
            nc.vector.tensor_reduce(sP, eP, axis=AX.X, op=ALU.add)
            rP = small.tile([P, G], F32)
            nc.vector.reciprocal(rP, sP)
            nc.vector.tensor_tensor(
                eP, eP, rP.unsqueeze(2).broadcast_to((P, G, NA)), ALU.mult
            )

            # y = clip(at,-10,10)*2.5 + 25, in place
            nc.vector.tensor_scalar(
                out=tat, in0=tat, scalar1=10.0, scalar2=-10.0, op0=ALU.min, op1=ALU.max
            )
            nc.vector.tensor_scalar(
                out=tat, in0=tat, scalar1=2.5, scalar2=25.0, op0=ALU.mult, op1=ALU.add
            )

            # exact bf16 split: y = hi + lo; ysp = [hi(51) | lo(51) | 1 | pad]
            ysp = mega.tile([P, G, 104], BF16)
            hi = ysp[:, :, 0:NA]
            lo = ysp[:, :, NA : 2 * NA]
            nc.vector.tensor_copy(hi, tat)  # f32 -> bf16 (round)
            nc.vector.tensor_tensor(lo, tat, hi, ALU.subtract)
            nc.vector.memset(ysp[:, :, 2 * NA : 2 * NA + 1], 1.0)

            accAll = small.tile([P, G], F32)

            # sQ = rowsum(x) * sum(p)
            sX = small.tile([P, G], F32)
            nc.vector.tensor_reduce(sX, x, axis=AX.X, op=ALU.add)
            sqAll = small.tile([P, G], F32)
            nc.vector.tensor_tensor(sqAll, sP, rP, ALU.mult)
            nc.vector.tensor_tensor(sqAll, sqAll, sX, ALU.mult)

            # ---- phase 2 ----
            for g in range(G):
                pst = psT.tile([NK, P], BF16)
                nc.tensor.transpose(pst, ysp[:, g, 0:NK], identb)
                lh = lhp.tile([NK, P], BF16)
                nc.scalar.copy(lh, pst)

                dabs = expp.tile([P, NJ, NI], BF16)
                dpsA = psDA.tile([P, 3, 512], F32)
                for ci, (j0, nj) in enumerate(CH_A):
                    nc.tensor.matmul(
                        dpsA[:, ci, 0 : nj * NI],
                        lhsT=lh,
                        rhs=selb[:, ci, 0 : nj * NI],
                        start=True,
                        stop=True,
                    )
                nc.scalar.activation(
                    dabs[:, 0:27, :].rearrange("p a b -> p (a b)").rearrange(
                        "p (c n) -> p c n", n=468
                    ),
                    dpsA[:, :, 0:468],
                    ACT.Abs,
                )
                dpsB = psDB.tile([P, 3, 512], F32)
                for ci, (j0, nj) in enumerate(CH_B):
                    nc.tensor.matmul(
                        dpsB[:, ci, 0 : nj * NI],
                        lhsT=lh,
                        rhs=selb[:, 3 + ci, 0 : nj * NI],
                        start=True,
                        stop=True,
                    )
                nc.scalar.activation(
                    dabs[:, 27:NJ, :].rearrange("p a b -> p (a b)").rearrange(
                        "p (c n) -> p c n", n=468
                    ),
                    dpsB[:, :, 0:468],
                    ACT.Abs,
                )

                # Q = p_j * x_i (bf16 out)
                pB = eP[:, g, :].unsqueeze(2).broadcast_to((P, NA, NI))
                xB = xe[:, g, :].unsqueeze(1).broadcast_to((P, NA, NI))
                q = expq.tile([P, NA, NI], BF16)
                eng = nc.vector if g % QDVE_EVERY == 0 else nc.gpsimd
                eng.tensor_tensor(q, pB, xB, ALU.mult)
                # acc = sum min(|d|,1) * Q  (DVE, bf16 2x, fp32 accum)
                nc.vector.scalar_tensor_tensor(
                    out=q,
                    in0=dabs[:, 0:NA, :],
                    scalar=1.0,
                    in1=q,
                    op0=ALU.min,
                    op1=ALU.mult,
                    accum_out=accAll[:, g : g + 1],
                )

            # ---- tail ----
            ce = small.tile([P, G], F32)
            nc.vector.tensor_tensor(ce, lse, sqAll, ALU.subtract)
            nc.vector.tensor_tensor(ce, ce, accAll, ALU.add)
            ctot = small.tile([P, 1], F32)
            nc.vector.tensor_reduce(ctot, ce, axis=AX.X, op=ALU.add)

            ps = psT.tile([1, 1], F32)
            nc.tensor.matmul(ps, lhsT=ctot, rhs=ones_col, start=True, stop=True)
            res = small.tile([1, 1], F32)
            nc.scalar.copy(res, ps)
            nc.sync.dma_start(out=out[:, :], in_=res)

    nc.compile()
    return nc


def kernel(logits_t, logits_tp1, atoms_target_t):
    if "nc" not in _CACHE:
        _CACHE["nc"] = _build()
    nc = _CACHE["nc"]

    logits_t = np.ascontiguousarray(logits_t, dtype=np.float32)
    logits_tp1 = np.ascontiguousarray(logits_tp1, dtype=np.float32)
    atoms_target_t = np.ascontiguousarray(atoms_target_t, dtype=np.float32)

    in_maps = []
    for k in range(N_CORES):
        sl = slice(k * R, (k + 1) * R)
        in_maps.append(
            {
                "logits_t": logits_t[sl],
                "logits_tp1": logits_tp1[sl],
                "atoms_target_t": atoms_target_t[sl],
            }
        )

    res = run_bass_kernel_spmd(nc, in_maps, core_ids=list(range(N_CORES)))
    total = sum(float(res.results[k]["out"][0, 0]) for k in range(N_CORES))
    return np.float32(total / BS)


# revision 22
# speedup vs baseline: 200.4084x; 1.0156x over previous
"""Trainium2 Bass kernel for nn_CategoricalRegressionLoss (C51 categorical
projection cross-entropy loss).

Math (per row b, 51 atoms, x = logits_t):
    p      = softmax(logits_tp1)
    y      = (clip(atoms_target_t, -10, 10) + 10) / 0.4     in [0, 50]
    G_b(y) = sum_i x[b,i] * relu(1 - |y - i|)     (PWL interp of x at y)
    ce[b]  = logsumexp(x[b,:]) - sum_j p[b,j] * G_b(y[b,j])
    out    = mean_b ce[b]

Dense evaluation over the expanded (j, i) grid:
    sum_j p_j G_b(y_j) = sum Q - sum min(|d|,1)*Q
    d = y_j - i,  Q = p_j * x_i,  sum Q = rowsum(x) * sum(p)

Engine split per 128-row group g:
    PE     d = y_j - i: transpose [y_hi|y_lo|1] (exact bf16 split of y), then
           one bf16 matmul per PSUM chunk against a selection matrix
           (rows j' place y at (j=j', i) blocks; last row adds -i).
    ACT    |d| PSUM->SBUF(bf16), exp/ln in phase 1
    DVE    fused min/mul/accumulate pass (bf16 2x) + 1/4 of Q builds
    GPSIMD Q = p_j * x_i outer products (3/4 of groups)

Sharding: pure data parallel, batch 65536 -> 8 cores x 8192 rows. Each core
emits a partial ce sum; host sums / batch size.
"""

import sys

sys.path.insert(0, "/opt/trn_rl_repo")

import numpy as np

import concourse.bacc as bacc
import concourse.tile as tile
import concourse.mybir as mybir
from concourse.bass_utils import run_bass_kernel_spmd
from concourse.masks import make_identity

N_CORES = 8
BS = 65536
NA = 51  # num atoms
NI = 52  # padded atom axis (even inner dim; extra atom has zero weight)
NK = 103  # contraction: 51 y_hi + 51 y_lo + ones row
R = BS // N_CORES  # rows per core
P = 128
G = R // P  # row-groups per core = 64

# PSUM d-chunks: 51 j-groups of 52 cols, ping-ponged over two 3-bank pools
CH_A = [(0, 9), (9, 9), (18, 9)]  # j 0..26
CH_B = [(27, 9), (36, 9), (45, 9)]  # j 27..53 (j 51..53 are zero pad)
NJ = 54  # padded j axis

F32 = mybir.dt.float32
BF16 = mybir.dt.bfloat16
I32 = mybir.dt.int32
ALU = mybir.AluOpType
ACT = mybir.ActivationFunctionType
AX = mybir.AxisListType

QDVE_EVERY = 3  # every 4th group's Q built on DVE, rest on GPSIMD

_CACHE = {}


def _build():
    nc = bacc.Bacc("TRN2", target_bir_lowering=False)

    lt = nc.dram_tensor("logits_t", (R, NA), F32, kind="ExternalInput")
    lp = nc.dram_tensor("logits_tp1", (R, NA), F32, kind="ExternalInput")
    at = nc.dram_tensor("atoms_target_t", (R, NA), F32, kind="ExternalInput")
    out = nc.dram_tensor("out", (1, 1), F32, kind="ExternalOutput")

    lt_r = lt.rearrange("(p g) a -> p g a", p=P)
    lp_r = lp.rearrange("(p g) a -> p g a", p=P)
    at_r = at.rearrange("(p g) a -> p g a", p=P)

    with tile.TileContext(nc) as tc:
        with (
            tc.tile_pool(name="mega", bufs=1) as mega,
            tc.tile_pool(name="small", bufs=1) as small,
            tc.tile_pool(name="lhp", bufs=3) as lhp,
            tc.tile_pool(name="expp", bufs=3) as expp,
            tc.tile_pool(name="expq", bufs=3) as expq,
            tc.tile_pool(name="psT", bufs=1, space="PSUM") as psT,
            tc.tile_pool(name="psDA", bufs=1, space="PSUM") as psDA,
            tc.tile_pool(name="psDB", bufs=1, space="PSUM") as psDB,
        ):
            # ---- constants ----
            identb = small.tile([P, P], BF16)
            make_identity(nc, identb)

            # selb[k, c, col]: for chunk c covering j'=9c..9c+8,
            # row j' (y_hi) and row 51+j' (y_lo) have ones on the 52-col block
            # of j'; row 102 has the -i pattern everywhere. Built with
            # full-tile iota + compares (partition-base-0 accesses only).
            NC6 = 6 * 468
            selb = small.tile([NK, 6, 512], BF16)
            nc.vector.memset(selb, 0.0)
            itA = small.tile([NK, 6, 468], I32)
            vA = itA.rearrange("p c (j i) -> p c j i", i=NI)
            nc.gpsimd.iota(
                vA, pattern=[[-9, 6], [-1, 9], [0, NI]], base=0, channel_multiplier=1
            )  # value = k - 9c - jl
            fA = small.tile([NK, 6, 468], F32)
            nc.vector.tensor_copy(fA, itA)
            itI = small.tile([NK, 6, 468], I32)
            vI = itI.rearrange("p c (j i) -> p c j i", i=NI)
            nc.gpsimd.iota(
                vI, pattern=[[0, 6], [0, 9], [-1, NI]], base=0, channel_multiplier=0
            )  # value = -i
            fI = small.tile([NK, 6, 468], F32)
            nc.vector.tensor_copy(fI, itI)
            itK = small.tile([NK, 6, 468], I32)
            nc.gpsimd.iota(
                itK[:, :, :], pattern=[[0, 6], [0, 468]], base=-102,
                channel_multiplier=1,
            )  # value = k - 102
            fK = small.tile([NK, 6, 468], F32)
            nc.vector.tensor_copy(fK, itK)
            # mh + ml: hi block (fA==0) + lo block (fA==51)
            mh = small.tile([NK, 6, 468], F32)
            nc.vector.tensor_scalar(
                out=mh, in0=fA, scalar1=0.0, scalar2=None, op0=ALU.is_equal
            )
            ml = small.tile([NK, 6, 468], F32)
            nc.vector.tensor_scalar(
                out=ml, in0=fA, scalar1=51.0, scalar2=None, op0=ALU.is_equal
            )
            nc.vector.tensor_tensor(mh, mh, ml, ALU.add)
            # row 102: (fK==0) * (-i)
            nc.vector.tensor_scalar(
                out=fK, in0=fK, scalar1=0.0, scalar2=None, op0=ALU.is_equal
            )
            nc.vector.tensor_tensor(fK, fK, fI, ALU.mult)
            nc.vector.tensor_tensor(
                selb[:, :, 0:468], mh, fK, ALU.add
            )
            ones_col = small.tile([P, 1], F32)
            nc.vector.memset(ones_col, 1.0)

            # ---- load inputs ----
            xe = mega.tile([P, G, NI], F32)  # logits_t, col 51 zero
            nc.vector.memset(xe[:, :, NA:NI], 0.0)
            nc.sync.dma_start(out=xe[:, :, 0:NA], in_=lt_r)
            tlp = mega.tile([P, G, NA], F32)
            nc.sync.dma_start(out=tlp, in_=lp_r)
            tat = mega.tile([P, G, NA], F32)
            nc.sync.dma_start(out=tat, in_=at_r)

            x = xe[:, :, 0:NA]

            # ---- phase 1 ----
            eT = mega.tile([P, G, NA], F32)
            nc.scalar.activation(eT, x, ACT.Exp)
            sT = small.tile([P, G], F32)
            nc.vector.tensor_reduce(sT, eT, axis=AX.X, op=ALU.add)
            lse = small.tile([P, G], F32)
            nc.scalar.activation(lse, sT, ACT.Ln)

            eP = mega.tile([P, G, NA], F32)
            nc.scalar.activation(eP, tlp, ACT.Exp)
            sP = small.tile([P, G], F32)
            nc.vector.tensor_reduce(sP, eP, axis=AX.X, op=ALU.add)
            rP = small.tile([P, G], F32)
            nc.vector.reciprocal(rP, sP)
            nc.vector.tensor_tensor(
                eP, eP, rP.unsqueeze(2).broadcast_to((P, G, NA)), ALU.mult
            )

            # y = clip(at,-10,10)*2.5 + 25, in place
            nc.vector.tensor_scalar(
                out=tat, in0=tat, scalar1=10.0, scalar2=-10.0, op0=ALU.min, op1=ALU.max
            )
            nc.vector.tensor_scalar(
                out=tat, in0=tat, scalar1=2.5, scalar2=25.0, op0=ALU.mult, op1=ALU.add
            )

            # exact bf16 split: y = hi + lo; ysp = [hi(51) | lo(51) | 1 | pad]
            ysp = mega.tile([P, G, 104], BF16)
            hi = ysp[:, :, 0:NA]
            lo = ysp[:, :, NA : 2 * NA]
            nc.vector.tensor_copy(hi, tat)  # f32 -> bf16 (round)
            nc.vector.tensor_tensor(lo, tat, hi, ALU.subtract)
            nc.vector.memset(ysp[:, :, 2 * NA : 2 * NA + 1], 1.0)

            accAll = small.tile([P, G], F32)

            # sQ = rowsum(x) * sum(p)
            sX = small.tile([P, G], F32)
            nc.vector.tensor_reduce(sX, x, axis=AX.X, op=ALU.add)
            sqAll = small.tile([P, G], F32)
            nc.vector.tensor_tensor(sqAll, sP, rP, ALU.mult)
            nc.vector.tensor_tensor(sqAll, sqAll, sX, ALU.mult)

            # ---- phase 2 ----
            for g in range(G):
                pst = psT.tile([NK, P], BF16)
                nc.tensor.transpose(pst, ysp[:, g, 0:NK], identb)
                lh = lhp.tile([NK, P], BF16)
                nc.scalar.copy(lh, pst)

                dabs = expp.tile([P, NJ, NI], BF16)
                dpsA = psDA.tile([P, 3, 512], F32)
                for ci, (j0, nj) in enumerate(CH_A):
                    nc.tensor.matmul(
                        dpsA[:, ci, 0 : nj * NI],
                        lhsT=lh,
                        rhs=selb[:, ci, 0 : nj * NI],
                        start=True,
                        stop=True,
                    )
                nc.scalar.activation(
                    dabs[:, 0:27, :].rearrange("p a b -> p (a b)").rearrange(
                        "p (c n) -> p c n", n=468
                    ),
                    dpsA[:, :, 0:468],
                    ACT.Abs,
                )
                dpsB = psDB.tile([P, 3, 512], F32)
                for ci, (j0, nj) in enumerate(CH_B):
                    nc.tensor.matmul(
                        dpsB[:, ci, 0 : nj * NI],
                        lhsT=lh,
                        rhs=selb[:, 3 + ci, 0 : nj * NI],
                        start=True,
                        stop=True,
                    )
                nc.scalar.activation(
                    dabs[:, 27:NJ, :].rearrange("p a b -> p (a b)").rearrange(
                        "p (c n) -> p c n", n=468
                    ),
                    dpsB[:, :, 0:468],
                    ACT.Abs,
                )

                # Q = p_j * x_i (bf16 out)
                pB = eP[:, g, :].unsqueeze(2).broadcast_to((P, NA, NI))
                xB = xe[:, g, :].unsqueeze(1).broadcast_to((P, NA, NI))
                q = expq.tile([P, NA, NI], BF16)
                eng = nc.vector if g % QDVE_EVERY == 0 else nc.gpsimd
                eng.tensor_tensor(q, pB, xB, ALU.mult)
                # acc = sum min(|d|,1) * Q  (DVE, bf16 2x, fp32 accum)
                nc.vector.scalar_tensor_tensor(
                    out=q,
                    in0=dabs[:, 0:NA, :],
                    scalar=1.0,
                    in1=q,
                    op0=ALU.min,
                    op1=ALU.mult,
                    accum_out=accAll[:, g : g + 1],
                )

            # ---- tail ----
            ce = small.tile([P, G], F32)
            nc.vector.tensor_tensor(ce, lse, sqAll, ALU.subtract)
            nc.vector.tensor_tensor(ce, ce, accAll, ALU.add)
            ctot = small.tile([P, 1], F32)
            nc.vector.tensor_reduce(ctot, ce, axis=AX.X, op=ALU.add)

            ps = psT.tile([1, 1], F32)
            nc.tensor.matmul(ps, lhsT=ctot, rhs=ones_col, start=True, stop=True)
            res = small.tile([1, 1], F32)
            nc.scalar.copy(res, ps)
            nc.sync.dma_start(out=out[:, :], in_=res)

    nc.compile()
    return nc


def kernel(logits_t, logits_tp1, atoms_target_t):
    if "nc" not in _CACHE:
        _CACHE["nc"] = _build()
    nc = _CACHE["nc"]

    logits_t = np.ascontiguousarray(logits_t, dtype=np.float32)
    logits_tp1 = np.ascontiguousarray(logits_tp1, dtype=np.float32)
    atoms_target_t = np.ascontiguousarray(atoms_target_t, dtype=np.float32)

    in_maps = []
    for k in range(N_CORES):
        sl = slice(k * R, (k + 1) * R)
        in_maps.append(
            {
                "logits_t": logits_t[sl],
                "logits_tp1": logits_tp1[sl],
                "atoms_target_t": atoms_target_t[sl],
            }
        )

    res = run_bass_kernel_spmd(nc, in_maps, core_ids=list(range(N_CORES)))
    total = sum(float(res.results[k]["out"][0, 0]) for k in range(N_CORES))
    return np.float32(total / BS)


# revision 32
# speedup vs baseline: 205.9274x; 1.0275x over previous
"""Trainium2 Bass kernel for nn_CategoricalRegressionLoss (C51 categorical
projection cross-entropy loss).

Math (per row b, 51 atoms, x = logits_t):
    p      = softmax(logits_tp1)
    y      = (clip(atoms_target_t, -10, 10) + 10) / 0.4     in [0, 50]
    G_b(y) = sum_i x[b,i] * relu(1 - |y - i|)     (PWL interp of x at y)
    ce[b]  = logsumexp(x[b,:]) - sum_j p[b,j] * G_b(y[b,j])
    out    = mean_b ce[b]

Dense evaluation over the expanded (j, i) grid:
    sum_j p_j G_b(y_j) = sum Q - sum min(|d|,1)*Q
    d = y_j - i,  Q = p_j * x_i,  sum Q = rowsum(x) * sum(p)

Engine split per 128-row group g:
    PE     d = y_j - i: transpose [y_hi|y_lo|1] (exact bf16 split of y), then
           one bf16 matmul per PSUM chunk against a selection matrix
           (rows j' place y at (j=j', i) blocks; last row adds -i).
    ACT    |d| PSUM->SBUF(bf16), exp/ln in phase 1
    DVE    fused min/mul/accumulate pass (paired groups) + ~1/3 of Q builds
    GPSIMD Q = p_j * x_i outer products (~2/3 of group-pairs) + phase-1
           clip/scale and softmax normalize

Sharding: pure data parallel, batch 65536 -> 8 cores x 8192 rows. Each core
emits a partial ce sum; host sums / batch size.
"""

import sys

sys.path.insert(0, "/opt/trn_rl_repo")

import numpy as np

import concourse.bacc as bacc
import concourse.tile as tile
import concourse.mybir as mybir
from concourse.bass_utils import run_bass_kernel_spmd
from concourse.masks import make_identity

N_CORES = 8
BS = 65536
NA = 51  # num atoms
NI = 52  # padded atom axis (even inner dim; extra atom has zero weight)
NK = 103  # contraction: 51 y_hi + 51 y_lo + ones row
R = BS // N_CORES  # rows per core
P = 128
G = R // P  # row-groups per core = 64

# PSUM d-chunks: 51 j-groups of 52 cols, ping-ponged over two 3-bank pools
CH_A = [(0, 9), (9, 9), (18, 9)]  # j 0..26
CH_B = [(27, 9), (36, 9), (45, 9)]  # j 27..53 (j 51..53 are zero pad)
NJ = 54  # padded j axis

F32 = mybir.dt.float32
BF16 = mybir.dt.bfloat16
I32 = mybir.dt.int32
ALU = mybir.AluOpType
ACT = mybir.ActivationFunctionType
AX = mybir.AxisListType

QDVE_EVERY = 3  # every 3rd group-pair's Q built on DVE, rest on GPSIMD

_CACHE = {}


def _build():
    nc = bacc.Bacc("TRN2", target_bir_lowering=False)

    lt = nc.dram_tensor("logits_t", (R, NA), F32, kind="ExternalInput")
    lp = nc.dram_tensor("logits_tp1", (R, NA), F32, kind="ExternalInput")
    at = nc.dram_tensor("atoms_target_t", (R, NA), F32, kind="ExternalInput")
    out = nc.dram_tensor("out", (1, 1), F32, kind="ExternalOutput")

    lt_r = lt.rearrange("(p g) a -> p g a", p=P)
    lp_r = lp.rearrange("(p g) a -> p g a", p=P)
    at_r = at.rearrange("(p g) a -> p g a", p=P)

    with tile.TileContext(nc) as tc:
        with (
            tc.tile_pool(name="mega", bufs=1) as mega,
            tc.tile_pool(name="small", bufs=1) as small,
            tc.tile_pool(name="lhp", bufs=4) as lhp,
            tc.tile_pool(name="expp", bufs=4) as expp,
            tc.tile_pool(name="expq", bufs=4) as expq,
            tc.tile_pool(name="psT", bufs=1, space="PSUM") as psT,
            tc.tile_pool(name="psDA", bufs=1, space="PSUM") as psDA,
            tc.tile_pool(name="psDB", bufs=1, space="PSUM") as psDB,
        ):
            # ---- constants ----
            identb = small.tile([P, P], BF16)
            make_identity(nc, identb)

            # selb[k, c, col]: for chunk c covering j'=9c..9c+8,
            # row j' (y_hi) and row 51+j' (y_lo) have ones on the 52-col block
            # of j'; row 102 has the -i pattern everywhere. Built with
            # full-tile iota + compares (partition-base-0 accesses only).
            selb = small.tile([NK, 6, 512], BF16)
            nc.vector.memset(selb, 0.0)
            with tc.tile_pool(name="scr", bufs=1) as scr:
                it = scr.tile([NK, 6, 468], I32)
                f = scr.tile([NK, 6, 468], F32)
                f2 = scr.tile([NK, 6, 468], F32)
                sF = scr.tile([NK, 6, 468], F32)
                nc.gpsimd.iota(
                    it.rearrange("p c (j i) -> p c j i", i=NI),
                    pattern=[[-9, 6], [-1, 9], [0, NI]], base=0,
                    channel_multiplier=1,
                )  # value = k - 9c - jl
                nc.vector.tensor_copy(f, it)
                nc.vector.tensor_scalar(
                    out=sF, in0=f, scalar1=0.0, scalar2=None, op0=ALU.is_equal
                )
                nc.vector.tensor_scalar(
                    out=f2, in0=f, scalar1=51.0, scalar2=None, op0=ALU.is_equal
                )
                nc.vector.tensor_tensor(sF, sF, f2, ALU.add)
                nc.gpsimd.iota(
                    it[:, :, :], pattern=[[0, 6], [0, 468]], base=-102,
                    channel_multiplier=1,
                )  # value = k - 102
                nc.vector.tensor_copy(f, it)
                nc.vector.tensor_scalar(
                    out=f, in0=f, scalar1=0.0, scalar2=None, op0=ALU.is_equal
                )
                nc.gpsimd.iota(
                    it.rearrange("p c (j i) -> p c j i", i=NI),
                    pattern=[[0, 6], [0, 9], [-1, NI]], base=0,
                    channel_multiplier=0,
                )  # value = -i
                nc.vector.tensor_copy(f2, it)
                nc.vector.tensor_tensor(f, f, f2, ALU.mult)
                nc.vector.tensor_tensor(sF, sF, f, ALU.add)
                nc.vector.tensor_copy(selb[:, :, 0:468], sF)
            ones_col = small.tile([P, 1], F32)
            nc.vector.memset(ones_col, 1.0)

            # ---- load inputs ----
            xe = mega.tile([P, G, NI], F32)  # logits_t, col 51 zero
            nc.vector.memset(xe[:, :, NA:NI], 0.0)
            nc.sync.dma_start(out=xe[:, :, 0:NA], in_=lt_r)
            tlp = mega.tile([P, G, NA], F32)
            nc.sync.dma_start(out=tlp, in_=lp_r)
            tat = mega.tile([P, G, NA], F32)
            nc.sync.dma_start(out=tat, in_=at_r)

            x = xe[:, :, 0:NA]

            # ---- phase 1 ----
            eT = mega.tile([P, G, NA], F32)
            nc.scalar.activation(eT, x, ACT.Exp)
            sT = small.tile([P, G], F32)
            nc.vector.tensor_reduce(sT, eT, axis=AX.X, op=ALU.add)
            lse = small.tile([P, G], F32)
            nc.scalar.activation(lse, sT, ACT.Ln)

            eP = tlp  # in-place exp; tlp not needed afterwards
            nc.scalar.activation(eP, tlp, ACT.Exp)
            sP = small.tile([P, G], F32)
            nc.vector.tensor_reduce(sP, eP, axis=AX.X, op=ALU.add)
            rP = small.tile([P, G], F32)
            nc.vector.reciprocal(rP, sP)
            nc.gpsimd.tensor_tensor(
                eP, eP, rP.unsqueeze(2).broadcast_to((P, G, NA)), ALU.mult
            )

            # y = clip(at,-10,10)*2.5 + 25, in place (GPSIMD)
            nc.gpsimd.tensor_scalar(
                out=tat, in0=tat, scalar1=10.0, scalar2=-10.0, op0=ALU.min, op1=ALU.max
            )
            nc.gpsimd.tensor_scalar(
                out=tat, in0=tat, scalar1=2.5, scalar2=25.0, op0=ALU.mult, op1=ALU.add
            )

            # exact bf16 split: y = hi + lo; ysp = [hi(51) | lo(51) | 1 | pad]
            ysp = mega.tile([P, G, 104], BF16)
            hi = ysp[:, :, 0:NA]
            lo = ysp[:, :, NA : 2 * NA]
            nc.vector.tensor_copy(hi, tat)  # f32 -> bf16 (round)
            nc.vector.tensor_tensor(lo, tat, hi, ALU.subtract)
            nc.vector.memset(ysp[:, :, 2 * NA : 2 * NA + 1], 1.0)


            # sQ = rowsum(x) * sum(p)
            sX = small.tile([P, G], F32)
            nc.vector.tensor_reduce(sX, x, axis=AX.X, op=ALU.add)
            sqAll = small.tile([P, G], F32)
            nc.vector.tensor_tensor(sqAll, sP, rP, ALU.mult)
            nc.vector.tensor_tensor(sqAll, sqAll, sX, ALU.mult)

            # ---- phase 2 (two row-groups per DVE/GPSIMD instruction) ----
            GP = G // 2
            accP = small.tile([P, GP], F32)
            for gp in range(GP):
                dabs = expp.tile([P, 2, NJ, NI], BF16)
                q = expq.tile([P, 2, NA, NI], BF16)
                for h in range(2):
                    g = 2 * gp + h
                    pst = psT.tile([NK, P], BF16)
                    nc.tensor.transpose(pst, ysp[:, g, 0:NK], identb)
                    lh = lhp.tile([NK, P], BF16)
                    nc.scalar.copy(lh, pst)

                    dpsA = psDA.tile([P, 3, 512], F32)
                    for ci, (j0, nj) in enumerate(CH_A):
                        nc.tensor.matmul(
                            dpsA[:, ci, 0 : nj * NI],
                            lhsT=lh,
                            rhs=selb[:, ci, 0 : nj * NI],
                            start=True,
                            stop=True,
                        )
                    nc.scalar.activation(
                        dabs[:, h, 0:27, :].rearrange("p a b -> p (a b)").rearrange(
                            "p (c n) -> p c n", n=468
                        ),
                        dpsA[:, :, 0:468],
                        ACT.Abs,
                    )
                    dpsB = psDB.tile([P, 3, 512], F32)
                    for ci, (j0, nj) in enumerate(CH_B):
                        nc.tensor.matmul(
                            dpsB[:, ci, 0 : nj * NI],
                            lhsT=lh,
                            rhs=selb[:, 3 + ci, 0 : nj * NI],
                            start=True,
                            stop=True,
                        )
                    nc.scalar.activation(
                        dabs[:, h, 27:NJ, :].rearrange("p a b -> p (a b)").rearrange(
                            "p (c n) -> p c n", n=468
                        ),
                        dpsB[:, :, 0:468],
                        ACT.Abs,
                    )

                # Q = p_j * x_i for both groups (bf16 out)
                g0 = 2 * gp
                pB = (
                    eP[:, g0 : g0 + 2, :]
                    .unsqueeze(3)
                    .broadcast_to((P, 2, NA, NI))
                )
                xB = (
                    xe[:, g0 : g0 + 2, :]
                    .unsqueeze(2)
                    .broadcast_to((P, 2, NA, NI))
                )
                eng = nc.vector if gp % QDVE_EVERY == 0 else nc.gpsimd
                eng.tensor_tensor(q, pB, xB, ALU.mult)
                # acc = sum min(|d|,1) * Q over both groups (fp32 accum)
                nc.vector.scalar_tensor_tensor(
                    out=q,
                    in0=dabs[:, :, 0:NA, :],
                    scalar=1.0,
                    in1=q,
                    op0=ALU.min,
                    op1=ALU.mult,
                    accum_out=accP[:, gp : gp + 1],
                )

            # ---- tail ----
            ce = small.tile([P, G], F32)
            nc.vector.tensor_tensor(ce, lse, sqAll, ALU.subtract)
            ctot = small.tile([P, 1], F32)
            nc.vector.tensor_reduce(ctot, ce, axis=AX.X, op=ALU.add)
            atot = small.tile([P, 1], F32)
            nc.vector.tensor_reduce(atot, accP, axis=AX.X, op=ALU.add)
            nc.vector.tensor_tensor(ctot, ctot, atot, ALU.add)

            ps = psT.tile([1, 1], F32)
            nc.tensor.matmul(ps, lhsT=ctot, rhs=ones_col, start=True, stop=True)
            res = small.tile([1, 1], F32)
            nc.scalar.copy(res, ps)
            nc.sync.dma_start(out=out[:, :], in_=res)

    nc.compile()
    return nc


def kernel(logits_t, logits_tp1, atoms_target_t):
    if "nc" not in _CACHE:
        _CACHE["nc"] = _build()
    nc = _CACHE["nc"]

    logits_t = np.ascontiguousarray(logits_t, dtype=np.float32)
    logits_tp1 = np.ascontiguousarray(logits_tp1, dtype=np.float32)
    atoms_target_t = np.ascontiguousarray(atoms_target_t, dtype=np.float32)

    in_maps = []
    for k in range(N_CORES):
        sl = slice(k * R, (k + 1) * R)
        in_maps.append(
            {
                "logits_t": logits_t[sl],
                "logits_tp1": logits_tp1[sl],
                "atoms_target_t": atoms_target_t[sl],
            }
        )

    res = run_bass_kernel_spmd(nc, in_maps, core_ids=list(range(N_CORES)))
    total = sum(float(res.results[k]["out"][0, 0]) for k in range(N_CORES))
    return np.float32(total / BS)
